# revision 1
# baseline (speedup 1.0000x reference)
"""3-layer GCN (100k nodes, 1.6M edges, 128->128->128->40) on 8 trn2 cores.

Self-contained harness kernel: kernel(**inputs) takes the FULL unsharded
inputs and returns the FULL [100000, 40] float32 output.

Strategy (1D node partition, edges sharded by dst, per the standard GCN
distribution):
  - nodes split contiguously across the 8 cores (12500 each, padded 12544);
    edges assigned to the core owning their dst.
  - per layer, each core computes the dense transform hp = (h @ W) * ns[row]
    on the PE (bf16 operands, f32 PSUM), stores rows as single bf16 (the
    2e-2 rel-err gate leaves plenty of headroom; layer 2 keeps a bf16 hi/lo
    split of its 64-wide output so the gather elem stays 256B).
  - the per-layer table replication is NBUK chunk-wise AllGathers (chunk =
    row range of every core's shard, chunk-major layout) issued from inside
    the previous layer's agg loop so they overlap compute; the LAST chunk is
    only 256 rows so the tail collective never gates the next layer.
  - aggregation: per (4-window group, bucket) a dma_gather pulls the 256B
    src rows into G (slots packed DENSE across the group's windows, 16-slot
    granularity); a one-hot S matrix (built on DVE from compile-time
    per-(window,column) dstl "variant" metadata via broadcast is_equal
    against iota) routes slots into a PSUM accumulator via bf16 matmuls.
    Boundary columns shared by two windows get one dstl variant per window
    (-1 lanes contribute zero), which is also how un-gathered tail garbage
    is masked.  Window epilogue applies nd / bias / relu on DVE+ACT and the
    transposed result feeds the next transform directly (no transposes).
  - gather indices are int16 (buckets of <=32768 rows), SBUF-resident for
    the whole run (replicated x8 across partition groups for the 8 Q7
    cores); counts are unioned across cores so all 8 share one SPMD program.
"""
import sys
sys.path.insert(0, '/opt/trn_rl_repo')

import math
import numpy as np

import concourse.bass as bass
import concourse.bacc as bacc
import concourse.tile as tile
import concourse.mybir as mybir
from concourse.bass_utils import run_bass_kernel_spmd

f32 = mybir.dt.float32
bf16 = mybir.dt.bfloat16
i16 = mybir.dt.int16

NC = 8
GW = 4  # windows per gather group


def _preprocess(src, dst, n_nodes):
    src = np.asarray(src).astype(np.int64)
    dst = np.asarray(dst).astype(np.int64)
    N = n_nodes
    assert N % NC == 0
    shard = N // NC
    NW = (shard + 127) // 128
    padshard = NW * 128
    NPAD = NC * padshard
    NBUK = max(1, math.ceil(NPAD / 32768))
    buksz = math.ceil(NPAD / NBUK / 128) * 128

    outdeg = np.bincount(src, minlength=N)
    indeg = np.bincount(dst, minlength=N)
    ns = (1.0 / np.sqrt(np.maximum(outdeg, 1))).astype(np.float32)
    nd = (1.0 / np.sqrt(np.maximum(indeg, 1))).astype(np.float32)

    # chunk-major layout: bucket b of the gather table = AllGather output of
    # per-core row chunk b (rows [b*CHS, (b+1)*CHS) of each core's padded
    # shard, concatenated core-major).  This lets each per-layer AllGather be
    # issued as NBUK small chunk collectives that overlap the agg loop.
    #
    # Slot packing is DENSE per (group, bucket): edges of all GW windows of a
    # group (sorted window-major) pack back-to-back; a 128-slot G column may
    # span two windows.  Disambiguation is via per-(window, column) dstl
    # "variant" columns: variant (w, col) holds each slot's dst-lane if the
    # slot belongs to window w, else -1 (is_equal -> 0, contributes nothing).
    # This cuts gather descriptors ~17% vs per-(window,bucket) 128-rounding.
    # uneven chunks: a small LEAD chunk so the first AllGather triggers and
    # drains early (startup), a small TAIL chunk so the last AllGather never
    # gates the next layer, middle chunks as big as int16 gather indices
    # allow (NC * chs <= 32768)
    max_chs = (32768 // NC) // 128 * 128
    lead, tail = 1024, 256
    if padshard > lead + tail + 128:
        mid_total = padshard - lead - tail
        n_mid = max(1, math.ceil(mid_total / max_chs))
        mid = [mid_total // n_mid // 128 * 128] * n_mid
        mid[0] += mid_total - sum(mid)
        chs = np.array([lead] + mid + [tail], dtype=np.int64)
    else:
        chs = np.array([padshard], dtype=np.int64)
    NBUK = len(chs)
    assert (chs > 0).all() and chs.max() <= max_chs and chs.sum() == padshard
    chstart = np.zeros(NBUK + 1, dtype=np.int64)
    chstart[1:] = np.cumsum(chs)
    buksz = [int(NC * c) for c in chs]
    assert max(buksz) <= 32768
    ecore = dst // shard
    NG = (NW + GW - 1) // GW

    cores = []
    cnt_gb = np.zeros((NC, NG, NBUK), dtype=np.int64)
    cnt_wb = np.zeros((NC, NW, NBUK), dtype=np.int64)
    for c in range(NC):
        m = ecore == c
        sc = src[m] // shard
        sr = src[m] % shard
        ld = dst[m] - c * shard
        w = ld >> 7
        slot = ld & 127
        b = np.searchsorted(chstart, sr, side='right') - 1
        reb = sc * chs[b] + (sr - chstart[b])
        g = w // GW
        order = np.lexsort((reb, w, b, g))
        g, b, w, slot, reb = g[order], b[order], w[order], slot[order], reb[order]
        kgb = g * NBUK + b
        cnt_gb[c] = np.bincount(kgb, minlength=NG * NBUK).reshape(NG, NBUK)
        cnt_wb[c] = np.bincount(w * NBUK + b, minlength=NW * NBUK).reshape(NW, NBUK)
        cores.append((kgb, w, b, slot, reb))

    NCOL = (cnt_gb.max(axis=0) + 127) // 128          # [NG, NBUK]
    NCOL[:, 0] = np.maximum(NCOL[:, 0], 1)

    # per-core start offset of window w's run inside its (g,b) call
    # (prefix over windows of the same group, same bucket)
    start_wb = np.zeros_like(cnt_wb)
    for g_ in range(NG):
        ws = range(g_ * GW, min((g_ + 1) * GW, NW))
        run = np.zeros((NC, NBUK), dtype=np.int64)
        for w_ in ws:
            start_wb[:, w_, :] = run
            run += cnt_wb[:, w_, :]

    # union column range of window w inside call (g,b), across cores
    fc = np.full((NW, NBUK), np.iinfo(np.int64).max, dtype=np.int64)
    lc = np.full((NW, NBUK), -1, dtype=np.int64)
    for c in range(NC):
        has = cnt_wb[c] > 0
        f = start_wb[c] >> 7
        l = (start_wb[c] + cnt_wb[c] - 1) >> 7
        fc[has] = np.minimum(fc[has], f[has])
        lc[has] = np.maximum(lc[has], l[has])
    nvar_wb = np.where(lc >= 0, lc - fc + 1, 0)       # [NW, NBUK]
    C_w = nvar_wb.sum(axis=1)
    assert (C_w >= 1).all()
    TOTCOL = int(C_w.sum())

    # window-major variant column layout (contiguous per window for S-build)
    vc_of = np.zeros((NW, NBUK), dtype=np.int64)
    colbase_w = np.zeros(NW, dtype=np.int64)
    acc = 0
    for w_ in range(NW):
        colbase_w[w_] = acc
        for b_ in range(NBUK):
            vc_of[w_, b_] = acc
            acc += int(nvar_wb[w_, b_])
    assert acc == TOTCOL

    # calls + per-window (variant, G-column) pairing
    groups = []
    call_off = np.zeros((NG, NBUK), dtype=np.int64)
    call_slab = np.zeros((NG, NBUK), dtype=np.int64)
    idx_off = 0
    for g_ in range(NG):
        ws = list(range(g_ * GW, min((g_ + 1) * GW, NW)))
        calls = []
        slabcol = 0
        for b_ in range(NBUK):
            n_cols = int(NCOL[g_, b_])
            if n_cols == 0:
                calls.append(None)
                continue
            call_off[g_, b_] = idx_off
            call_slab[g_, b_] = slabcol
            # descriptors rounded to 16 (idx packing), not 128: un-gathered
            # tail slots of the last column read stale SBUF, which is safe
            # ONLY because the kernel zero-fills all G pool buffers once at
            # startup (virgin SBUF bits can decode as NaN bf16, and
            # NaN * 0 = NaN would poison the one-hot matmul PSUM).
            maxcnt = int(cnt_gb[:, g_, b_].max())
            n_idx = min(n_cols * 128, max(16, ((maxcnt + 15) // 16) * 16))
            calls.append((b_, idx_off, n_idx, slabcol))
            idx_off += n_idx
            slabcol += n_cols
        groups.append((ws, calls, slabcol))
    TOTSLOT = idx_off
    assert TOTSLOT % 16 == 0

    gcol_of = []
    for g_, (ws, calls, _) in enumerate(groups):
        for w_ in ws:
            lst = []
            for b_ in range(NBUK):
                if calls[b_] is None:
                    continue
                for col in range(int(fc[w_, b_]), int(lc[w_, b_]) + 1):
                    lst.append(int(call_slab[g_, b_]) + col)
            gcol_of.append(lst)

    per_core = []
    for c in range(NC):
        kgb, w, b, slot, reb = cores[c]
        # dense slot position inside the (g,b) call
        nkey = NG * NBUK
        run_start = np.zeros(nkey, dtype=np.int64)
        run_start[1:] = np.cumsum(np.bincount(kgb, minlength=nkey))[:-1]
        pos_in_call = np.arange(len(kgb)) - run_start[kgb]
        g_arr = kgb // NBUK
        gslot = call_off[g_arr, b] + pos_in_call

        idx_flat = np.zeros(TOTSLOT, dtype=np.int16)
        idx_flat[gslot] = reb.astype(np.int16)

        # variant column of each edge + lane within column
        col_in_call = pos_in_call >> 7
        lane = pos_in_call & 127
        vc = vc_of[w, b] + (col_in_call - fc[w, b])
        assert (vc >= vc_of[w, b]).all()
        assert (vc < vc_of[w, b] + nvar_wb[w, b]).all()
        dstl_flat = np.full(TOTCOL * 128, -1.0, dtype=np.float32)
        dstl_flat[vc * 128 + lane] = slot.astype(np.float32)

        dstl2d = dstl_flat.reshape(TOTCOL, 128).T.copy()
        idx2d = np.tile(idx_flat.reshape(TOTSLOT // 16, 16).T, (8, 1)).copy()

        ns_sh = np.zeros(padshard, dtype=np.float32)
        nd_sh = np.zeros(padshard, dtype=np.float32)
        ns_sh[:shard] = ns[c * shard:(c + 1) * shard]
        nd_sh[:shard] = nd[c * shard:(c + 1) * shard]
        nscol = ns_sh.reshape(NW, 128).T.copy()
        ndcol = nd_sh.reshape(NW, 128).T.copy()
        ndrep = np.tile(nd_sh[None, :], (128, 1))

        per_core.append(dict(dstl=dstl2d, idx=idx2d, nscol=nscol, ndcol=ndcol,
                             ndrep=ndrep))

    struct = dict(N=N, shard=shard, NW=NW, padshard=padshard, NPAD=NPAD,
                  NBUK=NBUK, buksz=buksz, chs=chs, chstart=chstart, C_w=C_w,
                  TOTCOL=TOTCOL, TOTSLOT=TOTSLOT, colbase_w=colbase_w,
                  groups=groups, gcol_of=gcol_of)
    return struct, per_core


def _build_program(st, f_cls):
    NW, padshard, NPAD = st['NW'], st['padshard'], st['NPAD']
    NBUK, buksz = st['NBUK'], st['buksz']
    chs, chstart = st['chs'], st['chstart']
    C_w, TOTCOL, TOTSLOT = st['C_w'], st['TOTCOL'], st['TOTSLOT']
    colbase_w, groups, gcol_of = st['colbase_w'], st['groups'], st['gcol_of']
    shard = st['shard']
    fcp = 64 * ((f_cls + 63) // 64)

    nc = bacc.Bacc(None, target_bir_lowering=False,
                   num_swdge_queues=min(4, NBUK))

    featT_d = nc.dram_tensor("featT", [128, padshard], bf16, kind="ExternalInput")
    idx_d = nc.dram_tensor("idx16", [128, TOTSLOT // 16], i16, kind="ExternalInput")
    dstl_d = nc.dram_tensor("dstl", [128, TOTCOL], bf16, kind="ExternalInput")
    iota_d = nc.dram_tensor("iota", [128, 128], bf16, kind="ExternalInput")
    ndrep_d = nc.dram_tensor("ndrep", [128, padshard], f32, kind="ExternalInput")
    nscol_d = nc.dram_tensor("nscol", [128, NW], f32, kind="ExternalInput")
    ndcol_d = nc.dram_tensor("ndcol", [128, NW], f32, kind="ExternalInput")
    W0_d = nc.dram_tensor("W0", [128, 128], bf16, kind="ExternalInput")
    W1_d = nc.dram_tensor("W1", [128, 128], bf16, kind="ExternalInput")
    W2_d = nc.dram_tensor("W2p", [128, fcp], bf16, kind="ExternalInput")
    b0_d = nc.dram_tensor("b0c", [128, 1], f32, kind="ExternalInput")
    b1_d = nc.dram_tensor("b1c", [128, 1], f32, kind="ExternalInput")
    b2_d = nc.dram_tensor("b2rep", [128, fcp], f32, kind="ExternalInput")
    out_d = nc.dram_tensor("out", [shard, f_cls], f32, kind="ExternalOutput")

    hp0_own = [nc.dram_tensor(f"hp0_own{k}", [int(chs[k]), 128], bf16)
               for k in range(NBUK)]
    hp1_own = [nc.dram_tensor(f"hp1_own{k}", [int(chs[k]), 128], bf16)
               for k in range(NBUK)]
    hp2_own = [nc.dram_tensor(f"hp2_own{k}", [int(chs[k]), 2 * fcp], bf16)
               for k in range(NBUK)]
    hp0_full = [nc.dram_tensor(f"hp0_full{k}", [buksz[k], 128], bf16,
                               addr_space="Shared") for k in range(NBUK)]
    hp1_full = [nc.dram_tensor(f"hp1_full{k}", [buksz[k], 128], bf16,
                               addr_space="Shared") for k in range(NBUK)]
    hp2_full = [nc.dram_tensor(f"hp2_full{k}", [buksz[k], 2 * fcp], bf16,
                               addr_space="Shared") for k in range(NBUK)]

    rg = [list(range(NC))]
    # window after which per-core chunk k's transform rows are complete
    agw = {(int(chstart[k + 1]) - 1) // 128: k for k in range(NBUK)}

    with tile.TileContext(nc) as tc:
        with (
            tc.tile_pool(name="const", bufs=1) as cpool,
            tc.tile_pool(name="gpool", bufs=4) as gpool,
            tc.tile_pool(name="spool", bufs=4) as spool,
            tc.tile_pool(name="wpool", bufs=3) as wpool,
            tc.tile_pool(name="xpool", bufs=3) as xpool,
            tc.tile_pool(name="ftp", bufs=3) as ftp,
            tc.tile_pool(name="psA", bufs=3, space="PSUM") as psA,
            tc.tile_pool(name="psC", bufs=5, space="PSUM") as psC,
        ):
            sW0 = cpool.tile([128, 128], bf16); nc.sync.dma_start(sW0[:], W0_d[:])
            sW1 = cpool.tile([128, 128], bf16); nc.sync.dma_start(sW1[:], W1_d[:])
            sW2 = cpool.tile([128, fcp], bf16); nc.sync.dma_start(sW2[:], W2_d[:])
            sb0 = cpool.tile([128, 1], f32); nc.sync.dma_start(sb0[:], b0_d[:])
            sb1 = cpool.tile([128, 1], f32); nc.sync.dma_start(sb1[:], b1_d[:])
            sb2 = cpool.tile([128, fcp], f32); nc.sync.dma_start(sb2[:], b2_d[:])
            siota = cpool.tile([128, 128], bf16); nc.sync.dma_start(siota[:], iota_d[:])
            sdstl = cpool.tile([128, TOTCOL], bf16); nc.sync.dma_start(sdstl[:], dstl_d[:])
            snscol = cpool.tile([128, NW], f32); nc.sync.dma_start(snscol[:], nscol_d[:])
            sndcol = cpool.tile([128, NW], f32); nc.sync.dma_start(sndcol[:], ndcol_d[:])
            sndrep = cpool.tile([128, padshard], f32)
            nc.sync.dma_start(sndrep[:], ndrep_d[:])
            sidx = cpool.tile([128, TOTSLOT // 16], i16)
            nc.sync.dma_start(sidx[:], idx_d[:])

            def store_rows(hp_own_l, w, hp_tile):
                r0 = w * 128
                while r0 < (w + 1) * 128:
                    k = int(np.searchsorted(chstart, r0, side='right')) - 1
                    r1 = min((w + 1) * 128, int(chstart[k + 1]))
                    p0 = r0 - w * 128
                    nc.sync.dma_start(
                        hp_own_l[k][r0 - int(chstart[k]):r1 - int(chstart[k]), :],
                        hp_tile[p0:p0 + (r1 - r0), :])
                    r0 = r1

            def ag_chunk(hp_own_l, hp_full_l, k):
                nc.gpsimd.collective_compute(
                    "AllGather", mybir.AluOpType.bypass, rg,
                    ins=[hp_own_l[k][:, :]], outs=[hp_full_l[k][:, :]])

            def transform_single(w, lhsT_ap, sW, hp_own_l):
                ps2 = psA.tile([128, 128], f32)
                nc.tensor.matmul(ps2[:], lhsT_ap, sW, start=True, stop=True)
                hp = xpool.tile([128, 128], bf16, tag="hp")
                nc.scalar.activation(hp[:], ps2[:],
                                     mybir.ActivationFunctionType.Copy,
                                     scale=snscol[:, w:w + 1])
                store_rows(hp_own_l, w, hp)

            def transform_split(w, lhsT_ap, sW, fo, hp_own_l):
                ps2 = psA.tile([128, fo], f32)
                nc.tensor.matmul(ps2[:], lhsT_ap, sW, start=True, stop=True)
                tns = xpool.tile([128, fo], f32, tag="tns")
                nc.scalar.activation(tns[:], ps2[:],
                                     mybir.ActivationFunctionType.Copy,
                                     scale=snscol[:, w:w + 1])
                hp = xpool.tile([128, 2 * fo], bf16, tag="hp2")
                nc.scalar.activation(hp[:, 0:fo], tns[:],
                                     mybir.ActivationFunctionType.Copy)
                hif = xpool.tile([128, fo], f32, tag="hif")
                nc.scalar.activation(hif[:], hp[:, 0:fo],
                                     mybir.ActivationFunctionType.Copy)
                nc.vector.tensor_tensor(hp[:, fo:2 * fo], tns[:], hif[:],
                                        mybir.AluOpType.subtract)
                store_rows(hp_own_l, w, hp)

            FTB = 16
            for blk0 in range(0, NW, FTB):
                nwin = min(FTB, NW - blk0)
                ftb = ftp.tile([128, FTB * 128], bf16, tag="ftb")
                nc.sync.dma_start(ftb[:, 0:nwin * 128],
                                  featT_d[:, blk0 * 128:(blk0 + nwin) * 128])
                for w in range(blk0, blk0 + nwin):
                    o = (w - blk0) * 128
                    transform_single(w, ftb[:, o:o + 128], sW0[:], hp0_own)
                    if w in agw and agw[w] == 0:
                        ag_chunk(hp0_own, hp0_full, 0)

            LAG = 4
            HEAD = 4

            def agg_layer(hp_full_l, elem, layer, nxt=None, cur=None):
                trig = {}
                if nxt is not None:
                    for w_, k_ in agw.items():
                        trig.setdefault(w_ // GW + LAG, []).append(k_)
                done = set()

                def issue_call(G, call, gi):
                    b_, off, n_idx, slabcol = call
                    # rotate bucket->queue per group: buckets are unequal
                    # sizes, so a fixed b%4 mapping leaves one SWDGE queue
                    # at ~10% load while others carry ~30% — rotation gives
                    # every queue ~25% of packets
                    nc.gpsimd.dma_gather(
                        out_ap=G[:, slabcol:slabcol + (n_idx + 127) // 128, :],
                        in_ap=hp_full_l[b_][0:buksz[b_], :],
                        idxs_ap=sidx[:16, off // 16:(off + n_idx) // 16],
                        num_idxs=n_idx,
                        num_idxs_reg=n_idx,
                        elem_size=128,
                        single_packet=False,
                        queue_num=(b_ + gi) % min(4, NBUK),
                    )

                # cur = the CURRENT layer's (own, full) chunk tables whose
                # AllGathers 1..NBUK-1 are still untriggered (layer 0 only):
                # trigger them all up front — their store deps land before
                # AG_0 completes, and any gather placed ahead of them would
                # head-block the gpsimd queue on AG_0 and delay the triggers.
                if cur is not None:
                    for b_ in range(1, NBUK):
                        ag_chunk(cur[0], cur[1], b_)
                # zero-fill every G pool buffer once so un-gathered tail
                # slots never expose NaN-decoding virgin SBUF to the PE
                if layer == 0:
                    C_gmax = max(g[2] for g in groups)
                    for z in range(4):
                        Gz = gpool.tile([128, C_gmax, 128], bf16, tag="G",
                                        name="Gz")
                        nc.vector.memset(Gz[:], 0)
                # head groups: issue gathers bucket-major so early buckets'
                # gathers cover the tail AllGather chunk still in flight
                head = []
                for gi in range(min(HEAD, len(groups))):
                    C_g = groups[gi][2]
                    Gh = gpool.tile([128, C_g, 128], bf16, tag="G", name="Gh")
                    head.append(Gh)
                for b_ in range(NBUK):
                    for gi in range(len(head)):
                        call = groups[gi][1][b_]
                        if call is not None:
                            issue_call(head[gi], call, gi)

                for gi, (ws, calls, C_g) in enumerate(groups):
                    if nxt is not None:
                        for k_ in trig.get(gi, []):
                            ag_chunk(nxt[0], nxt[1], k_)
                            done.add(k_)
                    if gi < len(head):
                        G = head[gi]
                    else:
                        G = gpool.tile([128, C_g, 128], bf16, tag="G")
                        for call in calls:
                            if call is None:
                                continue
                            issue_call(G, call, gi)
                    for w in ws:
                        cw = int(C_w[w])
                        cb = int(colbase_w[w])
                        S = spool.tile([128, cw * 128], bf16, tag="S")
                        in0 = sdstl[:, cb:cb + cw].unsqueeze(2).broadcast_to([128, cw, 128])
                        in1 = siota[:, :].unsqueeze(1).broadcast_to([128, cw, 128])
                        nc.vector.tensor_tensor(
                            S[:, :].rearrange("p (c x) -> p c x", x=128),
                            in0, in1, mybir.AluOpType.is_equal)
                        if layer < 2:
                            ps = psC.tile([128, 128], f32, tag="psC")
                        else:
                            ps = psC.tile([128, elem], f32, tag="psC")
                        for k, gc in enumerate(gcol_of[w]):
                            first = k == 0
                            last = k == len(gcol_of[w]) - 1
                            Sk = S[:, k * 128:(k + 1) * 128]
                            if layer < 2:
                                nc.tensor.matmul(ps[:], G[:, gc, :], Sk,
                                                 start=first, stop=last)
                            else:
                                nc.tensor.matmul(ps[:], Sk, G[:, gc, 0:elem],
                                                 start=first, stop=False)
                                nc.tensor.matmul(ps[:], Sk, G[:, gc, elem:2 * elem],
                                                 start=False, stop=last)
                        if layer < 2:
                            t = xpool.tile([128, 128], f32, tag="tagg")
                            nc.vector.tensor_tensor(
                                t[:], ps[:], sndrep[:, w * 128:(w + 1) * 128],
                                mybir.AluOpType.mult)
                            hsT = wpool.tile([128, 128], bf16, tag="hsT")
                            bias = sb0 if layer == 0 else sb1
                            nc.scalar.activation(hsT[:], t[:],
                                                 mybir.ActivationFunctionType.Relu,
                                                 bias=bias[:])
                            if layer == 0:
                                transform_single(w, hsT[:], sW1[:], hp1_own)
                            else:
                                transform_split(w, hsT[:], sW2[:], fcp, hp2_own)
                        else:
                            t = xpool.tile([128, elem], f32, tag="tout")
                            nc.scalar.activation(t[:], ps[:],
                                                 mybir.ActivationFunctionType.Copy,
                                                 scale=sndcol[:, w:w + 1])
                            o = xpool.tile([128, elem], f32, tag="oout")
                            nc.vector.tensor_tensor(o[:], t[:], sb2[:, 0:elem],
                                                    mybir.AluOpType.add)
                            rows = min(128, shard - w * 128)
                            nc.sync.dma_start(out_d[w * 128:w * 128 + rows, :],
                                              o[:rows, 0:f_cls])
                if nxt is not None:
                    for k_ in range(NBUK):
                        if k_ not in done:
                            ag_chunk(nxt[0], nxt[1], k_)

            agg_layer(hp0_full, 128, 0, nxt=(hp1_own, hp1_full),
                      cur=(hp0_own, hp0_full))
            agg_layer(hp1_full, 128, 1, nxt=(hp2_own, hp2_full))
            agg_layer(hp2_full, fcp, 2)

    nc.compile()
    return nc


_cache = {}


def kernel(feat, src, dst, W0, b0, W1, b1, W2, b2):
    import ml_dtypes
    feat = np.ascontiguousarray(feat, dtype=np.float32)
    N = feat.shape[0]
    f_cls = np.asarray(W2).shape[1]
    fcp = 64 * ((f_cls + 63) // 64)

    key = (N, hash(np.asarray(src).tobytes()), hash(np.asarray(dst).tobytes()))
    if key in _cache:
        st, per_core, nc_prog = _cache[key]
    else:
        st, per_core = _preprocess(src, dst, N)
        nc_prog = _build_program(st, f_cls)
        _cache[key] = (st, per_core, nc_prog)

    shard, padshard, NW = st['shard'], st['padshard'], st['NW']
    iota = np.tile(np.arange(128, dtype=np.float32), (128, 1))
    W2p = np.zeros((128, fcp), dtype=np.float32)
    W2p[:, :f_cls] = np.asarray(W2, dtype=np.float32)
    b2rep = np.zeros((128, fcp), dtype=np.float32)
    b2rep[:, :f_cls] = np.asarray(b2, dtype=np.float32)[None, :]
    bfv = lambda a: np.ascontiguousarray(a).astype(ml_dtypes.bfloat16)

    in_maps = []
    for c in range(NC):
        pc = per_core[c]
        featT = np.zeros((128, padshard), dtype=np.float32)
        featT[:, :shard] = feat[c * shard:(c + 1) * shard, :].T
        in_maps.append({
            "featT": bfv(featT),
            "idx16": pc['idx'],
            "dstl": bfv(pc['dstl']),
            "iota": bfv(iota),
            "ndrep": pc['ndrep'],
            "nscol": pc['nscol'],
            "ndcol": pc['ndcol'],
            "W0": bfv(np.asarray(W0, dtype=np.float32)),
            "W1": bfv(np.asarray(W1, dtype=np.float32)),
            "W2p": bfv(W2p),
            "b0c": np.asarray(b0, dtype=np.float32).reshape(128, 1),
            "b1c": np.asarray(b1, dtype=np.float32).reshape(128, 1),
            "b2rep": b2rep,
        })

    import os
    trace = os.environ.get("GCN_TRACE") == "1"
    res = run_bass_kernel_spmd(nc_prog, in_maps, core_ids=list(range(NC)),
                               trace=trace)
    global last_results
    last_results = res
    out = np.concatenate([res.results[c]["out"] for c in range(NC)], axis=0)
    return np.ascontiguousarray(out, dtype=np.float32)


last_results = None

